# revision 5
# baseline (speedup 1.0000x reference)
"""Multi-head self-attention (RoPE, causal) TRN2 Bass kernel — fused v2.

Problem: B=4, S=2048, D=1024, H=16, Dh=64, fp32 in/out.

Sharding (8 cores): DP=4 over batch x TP=2 over heads (Megatron-style).
Core c handles batch c//2 with heads (c%2)*8 .. (c%2)*8+7 and produces a
partial output [D, S]; the host sums the two TP partials per batch (the
all-reduce after out_projection) and transposes back.

v2 changes vs the 3-phase baseline (389us):
  - Everything bf16 on SBUF (same 1 cyc/row PE speed as f32r, 2x DVE modes,
    half the SBUF/DMA footprint, FWL on LDWEIGHTS). PSUM stays f32.
  - Single fused pipeline: per 512-token tile ts, emit QKV projection(ts),
    then causal attention row i=ts (kv tiles 0..ts), then out-projection of
    row ts. Tile's cost-model scheduler interleaves across sections, so the
    PE never idles long enough to drop the HAM clock gate, and the
    ACT-bound attention inner loop is overlapped with QKV/out-proj matmuls.
  - ACT (scalar engine) runs ONLY the softmax exps (the true ACT floor);
    all copies are pinned to DVE explicitly.
  - e-major W layout + per-d X chunks so the first matmul can start ~1us in.
  - Normalization: per-2-pair-group reciprocal, pair-packed one-hot
    broadcast matmuls, dedicated PSUM rotation, den gathers issued per-pair.
"""

import sys

for _p in ("/opt/trn_rl_repo", "/root/.axon_site/_ro/trn_rl_repo"):
    if _p not in sys.path:
        sys.path.insert(0, _p)

import ml_dtypes
import numpy as np

import concourse.bacc as bacc
import concourse.bass_utils as bass_utils
import concourse.mybir as mybir
import concourse.tile as tile
from concourse.bass_utils import run_bass_kernel_spmd

F32 = mybir.dt.float32
BF16 = mybir.dt.bfloat16
EXP = mybir.ActivationFunctionType.Exp
BF = ml_dtypes.bfloat16

B, S, D = 4, 2048, 1024
H, DH = 16, 64
THETA = 10000.0
NCORES, TP, HLOC = 8, 2, 8
NPAIR = HLOC // 2
NT = S // 512                        # 4 tiles of 512 tokens
ND = D // 128                        # 8 contraction chunks
SCALE = 1.0 / 8.0                    # 1/sqrt(DH)

_PROGRAM = None


def _build_program():
    nc = bacc.Bacc(None)

    xT_d = nc.dram_tensor("xT", [D, S], BF16, kind="ExternalInput")
    weT_d = nc.dram_tensor("weT", [8, 128, 1024], BF16, kind="ExternalInput")
    wvT_d = nc.dram_tensor("wvT", [8, 128, 512], BF16, kind="ExternalInput")
    woT_d = nc.dram_tensor("woT", [NPAIR, 128, D], BF16, kind="ExternalInput")
    cos_d = nc.dram_tensor("cosT", [128, S], BF16, kind="ExternalInput")
    sin_d = nc.dram_tensor("sinT", [128, S], BF16, kind="ExternalInput")
    mask_d = nc.dram_tensor("mask", [128, 128], BF16, kind="ExternalInput")
    sel_d = nc.dram_tensor("sel", [4, 256], BF16, kind="ExternalInput")
    out_d = nc.dram_tensor("out", [D, S], F32, kind="ExternalOutput")

    with tile.TileContext(nc) as tc:
        with (
            tc.tile_pool(name="const", bufs=1) as constp,
            tc.tile_pool(name="wpool", bufs=1) as wpool,
            tc.tile_pool(name="qkpool", bufs=1) as qkpool,
            tc.tile_pool(name="vpool", bufs=1) as vpool,
            tc.tile_pool(name="xpool", bufs=1) as xpool,
            tc.tile_pool(name="rpool", bufs=1) as rpool,
            tc.tile_pool(name="ptpool", bufs=1) as ptpool,
            tc.tile_pool(name="nrmpool", bufs=1) as nrmpool,
            tc.tile_pool(name="otpool", bufs=1) as otpool,
            tc.tile_pool(name="stp", bufs=1, space="PSUM") as stp,
            tc.tile_pool(name="cxp", bufs=1, space="PSUM") as cxp,
            tc.tile_pool(name="genp", bufs=1, space="PSUM") as genp,
        ):
            # ---- persistent weights / constants (parallel DMA queues) ----
            we = [wpool.tile([128, 1024], BF16, name=f"we{e}") for e in range(8)]
            wv = [wpool.tile([128, 512], BF16, name=f"wv{d}") for d in range(ND)]
            wo = [wpool.tile([128, D], BF16, name=f"wo{p}") for p in range(NPAIR)]
            for e in range(8):
                nc.gpsimd.dma_start(we[e][:], weT_d[e])
            for d in range(ND):
                nc.gpsimd.dma_start(wv[d][:], wvT_d[d])
            mask_sb = constp.tile([128, 128], BF16)
            sel_sb = constp.tile([4, 256], BF16)
            nc.gpsimd.dma_start(mask_sb[:], mask_d[:])
            nc.gpsimd.dma_start(sel_sb[:], sel_d[:])
            for p in range(NPAIR):
                nc.gpsimd.dma_start(wo[p][:], woT_d[p])

            qt = [qkpool.tile([128, S], BF16, name=f"qt{p}") for p in range(NPAIR)]
            kt = [qkpool.tile([128, S], BF16, name=f"kt{p}") for p in range(NPAIR)]
            vt = [vpool.tile([128, HLOC, DH + 1], BF16, name=f"v{t}")
                  for t in range(4 * NT)]
            for t in range(4 * NT):
                nc.gpsimd.memset(vt[t][:, :, DH:DH + 1], 1.0)

            def emit_loads(ts):
                tsl = slice(ts * 512, (ts + 1) * 512)
                xa = xpool.tile([128, ND, 512], BF16, tag="x", bufs=2,
                                name=f"xa{ts}")
                for d in range(ND):
                    # first tile: split across sync + scalar queues (no exp
                    # traffic exists yet) so the first chain starts ~1us in
                    eng = nc.scalar if (ts == 0 and d % 2 == 1) else nc.sync
                    eng.dma_start(xa[:, d, :], xT_d[d * 128:(d + 1) * 128, tsl])
                cos_sb = rpool.tile([128, 512], BF16, tag="cos", bufs=2,
                                    name=f"cos{ts}")
                sin_sb = rpool.tile([128, 512], BF16, tag="sin", bufs=2,
                                    name=f"sin{ts}")
                nc.sync.dma_start(cos_sb[:], cos_d[:, tsl])
                nc.sync.dma_start(sin_sb[:], sin_d[:, tsl])
                return xa, cos_sb, sin_sb

            deng = None
            loads = emit_loads(0)
            for ts in range(NT):
                tsl = slice(ts * 512, (ts + 1) * 512)
                i = ts
                isl = tsl
                xa, cos_sb, sin_sb = loads

                # Q^T / K^T e-chunks with RoPE
                for e in range(8):
                    ps = genp.tile([128, 512], F32, tag="gen", bufs=2)
                    for d in range(ND):
                        nc.tensor.matmul(
                            ps[:], we[e][:, d * 128:(d + 1) * 128], xa[:, d, :],
                            start=(d == 0), stop=(d == ND - 1),
                        )
                    dst = qt[e] if e < 4 else kt[e - 4]
                    nc.vector.tensor_copy(dst[:, tsl], ps[:])
                    # RoPE: quadrant swap via SBUF->SBUF DMA, then 2 muls + add
                    sw = rpool.tile([128, 512], BF16, tag="sw", bufs=3)
                    for qd in range(4):
                        sq = qd ^ 1
                        nc.gpsimd.dma_start(
                            sw[qd * 32:(qd + 1) * 32, :],
                            dst[sq * 32:(sq + 1) * 32, tsl],
                        )
                    t1 = rpool.tile([128, 512], BF16, tag="t1", bufs=3)
                    nc.vector.tensor_mul(t1[:], dst[:, tsl], cos_sb[:])
                    nc.vector.tensor_mul(sw[:], sw[:], sin_sb[:])
                    nc.vector.tensor_add(dst[:, tsl], t1[:], sw[:])

                # V chunks (natural [t, h, dv] layout, ones column appended)
                for tq0 in range(4):
                    tq = ts * 4 + tq0
                    psv = genp.tile([128, 512], F32, tag="gen", bufs=2)
                    for d in range(ND):
                        nc.tensor.matmul(
                            psv[:], xa[:, d, tq0 * 128:(tq0 + 1) * 128], wv[d][:],
                            start=(d == 0), stop=(d == ND - 1),
                        )
                    nc.vector.tensor_copy(
                        vt[tq][:, :, 0:DH],
                        psv.rearrange("p (h d) -> p h d", h=HLOC),
                    )

                if ts + 1 < NT:
                    loads = emit_loads(ts + 1)

                # ---------------- attention row i=ts ----------------
                nj = 4 * i + 4
                for p in range(NPAIR):
                    cxa = cxp.tile([65, 512], F32, tag="cxa", bufs=1,
                                   name=f"cxa{i}_{p}")
                    cxb = cxp.tile([65, 512], F32, tag="cxb", bufs=1,
                                   name=f"cxb{i}_{p}")
                    for j in range(nj):
                        lo = max(0, 128 * j - 512 * i)
                        qsl = slice(512 * i + lo, 512 * (i + 1))
                        ksl = slice(j * 128, (j + 1) * 128)
                        st = stp.tile([128, 2, 512], F32, tag="st", bufs=2)
                        nc.tensor.matmul(
                            st[:, 0, lo:512], kt[p][0:64, ksl],
                            qt[p][0:64, qsl], tile_position=(0, 0),
                        )
                        nc.tensor.matmul(
                            st[:, 1, lo:512], kt[p][64:128, ksl],
                            qt[p][64:128, qsl], tile_position=(64, 0),
                        )
                        pt = ptpool.tile([128, 2, 512], BF16, tag="pt", bufs=6)
                        nc.scalar.activation(
                            pt[:, :, lo:512], st[:, :, lo:512], EXP, scale=SCALE)
                        if lo == 128 * j - 512 * i:  # block straddles diagonal
                            nc.vector.tensor_mul(
                                pt[:, :, lo:lo + 128],
                                pt[:, :, lo:lo + 128],
                                mask_sb[:, None, :].to_broadcast([128, 2, 128]),
                            )
                        nc.tensor.matmul(
                            cxa[:, lo:512], vt[j][:, 2 * p, :],
                            pt[:, 0, lo:512],
                            start=(j == 0), stop=(j == nj - 1),
                        )
                        nc.tensor.matmul(
                            cxb[:, lo:512], vt[j][:, 2 * p + 1, :],
                            pt[:, 1, lo:512],
                            start=(j == 0), stop=(j == nj - 1),
                        )

                    # stash (row 64 = softmax denominator), gather, repack
                    g, q = p // 2, p % 2
                    if q == 0:
                        deng = nrmpool.tile([4, 512], BF16, tag="deng", bufs=2,
                                            name=f"deng{i}_{g}")
                    ctxb_sb = nrmpool.tile([65, 512], BF16, tag="ctxb", bufs=2)
                    nc.vector.tensor_copy(qt[p][0:65, isl], cxa[:])
                    nc.vector.tensor_copy(ctxb_sb[:], cxb[:])
                    nc.gpsimd.dma_start(deng[2 * q:2 * q + 1, :],
                                        qt[p][64:65, isl])
                    nc.gpsimd.dma_start(
                        deng[2 * q + 1:2 * q + 2, :], ctxb_sb[64:65, :])
                    nc.gpsimd.dma_start(qt[p][64:128, isl], ctxb_sb[0:64, :])

                    if q == 1:  # pairs 2g, 2g+1 stashed -> normalize them
                        denf = nrmpool.tile([4, 512], F32, tag="denf", bufs=2)
                        nc.vector.tensor_copy(denf[:], deng[:])
                        rec = nrmpool.tile([4, 512], F32, tag="rec", bufs=2)
                        nc.vector.reciprocal_approx_fast(rec[:], denf[:])
                        recr = nrmpool.tile([4, 512], BF16, tag="recr", bufs=2)
                        nc.vector.tensor_copy(recr[:], rec[:])
                        for pp in (2 * g, 2 * g + 1):
                            qq = pp % 2
                            bc = genp.tile([128, 512], F32, tag="gen", bufs=2,
                                           name=f"bc{i}_{pp}")
                            nc.tensor.matmul(
                                bc[:], sel_sb[:, qq * 128:(qq + 1) * 128],
                                recr[:])
                            bcs = nrmpool.tile([128, 512], BF16, tag="bcs",
                                               bufs=2)
                            nc.vector.tensor_copy(bcs[:], bc[:])
                            nc.vector.tensor_mul(
                                qt[pp][:, isl], qt[pp][:, isl], bcs[:])

                # ---------------- out-projection row i=ts ----------------
                for ec in range(D // 128):
                    ecs = slice(ec * 128, (ec + 1) * 128)
                    pso = genp.tile([128, 512], F32, tag="gen", bufs=2,
                                    name=f"pso{i}_{ec}")
                    for p in range(NPAIR):
                        nc.tensor.matmul(
                            pso[:], wo[p][:, ecs], qt[p][:, isl],
                            start=(p == 0), stop=(p == NPAIR - 1),
                        )
                    ot = otpool.tile([128, 512], F32, tag="ot", bufs=3)
                    if i < 3:
                        nc.vector.tensor_copy(ot[:], pso[:])
                    else:
                        nc.scalar.copy(ot[:], pso[:])
                    nc.gpsimd.dma_start(out_d[ecs, isl], ot[:])

    nc.compile()
    return nc


def _get_program():
    global _PROGRAM
    if _PROGRAM is None:
        _PROGRAM = _build_program()
    return _PROGRAM


def _prep_in_maps(in_features, token_positions, W_qkv, W_out):
    in_features = np.asarray(in_features, dtype=np.float32)
    token_positions = np.asarray(token_positions)
    W_qkv = np.asarray(W_qkv, dtype=np.float32)
    W_out = np.asarray(W_out, dtype=np.float32)

    # RoPE pair permutation: [x0 of freq 0..31 | x1 of freq 0..31]
    perm = np.concatenate([np.arange(0, DH, 2), np.arange(1, DH, 2)])

    weT, wvT, woT = [], [], []
    for tp in range(TP):
        rows = []
        for sect in range(2):  # Q, K (permuted)
            for h in range(HLOC):
                g = tp * HLOC + h
                rows.append(W_qkv[sect * D + g * DH + perm])
        for h in range(HLOC):  # V natural
            g = tp * HLOC + h
            rows.append(W_qkv[2 * D + g * DH:2 * D + (g + 1) * DH])
        Wl = np.concatenate(rows, axis=0)          # [1536, 1024]
        wqkvT = np.ascontiguousarray(Wl.T)         # [1024, 1536]
        tmp = wqkvT[:, 0:1024].reshape(8, 128, 8, 128)        # [d, p, e, c]
        weT.append(np.ascontiguousarray(
            tmp.transpose(2, 1, 0, 3).reshape(8, 128, 1024).astype(BF)))
        wvT.append(np.ascontiguousarray(
            wqkvT[:, 1024:1536].reshape(8, 128, 512).astype(BF)))
        woT.append(np.ascontiguousarray(np.stack(
            [np.concatenate([
                W_out[:, (tp * HLOC + 2 * p) * DH:(tp * HLOC + 2 * p + 1) * DH].T,
                W_out[:, (tp * HLOC + 2 * p + 1) * DH:(tp * HLOC + 2 * p + 2) * DH].T,
            ], axis=0) for p in range(NPAIR)])).astype(BF))

    half = DH // 2
    inv_freq = (THETA ** (-2.0 * np.arange(half, dtype=np.float32) / DH)
                ).astype(np.float32)
    ang = token_positions.astype(np.float32)[:, None] * inv_freq[None, :]
    cos_t = np.cos(ang).T.astype(np.float32)  # [32, S]
    sin_t = np.sin(ang).T.astype(np.float32)
    cos128 = np.ascontiguousarray(np.tile(cos_t, (4, 1))).astype(BF)
    sin128 = np.ascontiguousarray(
        np.tile(np.concatenate([-sin_t, sin_t], axis=0), (2, 1))).astype(BF)
    # mask[kv, q] = 1 iff kv <= q (scores stored transposed: [kv, q])
    mask128 = np.triu(np.ones((128, 128), dtype=np.float32)).astype(BF)
    # sel[r, q*128 + c] = 1 iff r == 2q + (c >= 64): pair-packed recip bcast
    sel = np.zeros((4, 256), dtype=np.float32)
    for q in range(2):
        sel[2 * q, q * 128:q * 128 + 64] = 1.0
        sel[2 * q + 1, q * 128 + 64:(q + 1) * 128] = 1.0
    sel = sel.astype(BF)

    in_maps = []
    for c in range(NCORES):
        b, tp = c // 2, c % 2
        in_maps.append({
            "xT": np.ascontiguousarray(in_features[b].T).astype(BF),
            "weT": weT[tp],
            "wvT": wvT[tp],
            "woT": woT[tp],
            "cosT": cos128,
            "sinT": sin128,
            "mask": mask128,
            "sel": sel,
        })
    return in_maps


def run(in_features, token_positions, W_qkv, W_out, **spmd_kwargs):
    """Run the kernel; returns (output [B,S,D] f32, BassKernelResults)."""
    in_maps = _prep_in_maps(in_features, token_positions, W_qkv, W_out)
    nc = _get_program()
    res = run_bass_kernel_spmd(nc, in_maps, core_ids=list(range(NCORES)),
                               **spmd_kwargs)
    outs = [res.results[c]["out"] for c in range(NCORES)]
    full = np.stack([(outs[2 * b] + outs[2 * b + 1]).T for b in range(B)])
    return full.astype(np.float32), res


def kernel(in_features, token_positions, W_qkv, W_out):
    out, _ = run(in_features, token_positions, W_qkv, W_out)
    return out


# revision 6
# speedup vs baseline: 1.0184x; 1.0184x over previous
"""Multi-head self-attention (RoPE, causal) TRN2 Bass kernel — fused v2.

Problem: B=4, S=2048, D=1024, H=16, Dh=64, fp32 in/out.

Sharding (8 cores): DP=4 over batch x TP=2 over heads (Megatron-style).
Core c handles batch c//2 with heads (c%2)*8 .. (c%2)*8+7 and produces a
partial output [D, S]; the host sums the two TP partials per batch (the
all-reduce after out_projection) and transposes back.

v2 changes vs the 3-phase baseline (389us):
  - Everything bf16 on SBUF (same 1 cyc/row PE speed as f32r, 2x DVE modes,
    half the SBUF/DMA footprint, FWL on LDWEIGHTS). PSUM stays f32.
  - Single fused pipeline: per 512-token tile ts, emit QKV projection(ts),
    then causal attention row i=ts (kv tiles 0..ts), then out-projection of
    row ts. Tile's cost-model scheduler interleaves across sections, so the
    PE never idles long enough to drop the HAM clock gate, and the
    ACT-bound attention inner loop is overlapped with QKV/out-proj matmuls.
  - ACT (scalar engine) runs ONLY the softmax exps (the true ACT floor);
    all copies are pinned to DVE explicitly.
  - e-major W layout + per-d X chunks so the first matmul can start ~1us in.
  - Normalization: per-2-pair-group reciprocal, pair-packed one-hot
    broadcast matmuls, dedicated PSUM rotation, den gathers issued per-pair.
"""

import sys

for _p in ("/opt/trn_rl_repo", "/root/.axon_site/_ro/trn_rl_repo"):
    if _p not in sys.path:
        sys.path.insert(0, _p)

import ml_dtypes
import numpy as np

import concourse.bacc as bacc
import concourse.bass_utils as bass_utils
import concourse.mybir as mybir
import concourse.tile as tile
from concourse.bass_utils import run_bass_kernel_spmd

F32 = mybir.dt.float32
BF16 = mybir.dt.bfloat16
EXP = mybir.ActivationFunctionType.Exp
BF = ml_dtypes.bfloat16

B, S, D = 4, 2048, 1024
H, DH = 16, 64
THETA = 10000.0
NCORES, TP, HLOC = 8, 2, 8
NPAIR = HLOC // 2
NT = S // 512                        # 4 tiles of 512 tokens
ND = D // 128                        # 8 contraction chunks
SCALE = 1.0 / 8.0                    # 1/sqrt(DH)

_PROGRAM = None


def _build_program():
    nc = bacc.Bacc(None)

    xT_d = nc.dram_tensor("xT", [D, S], BF16, kind="ExternalInput")
    weT_d = nc.dram_tensor("weT", [8, 128, 1024], BF16, kind="ExternalInput")
    wvT_d = nc.dram_tensor("wvT", [8, 128, 512], BF16, kind="ExternalInput")
    woT_d = nc.dram_tensor("woT", [NPAIR, 128, D], BF16, kind="ExternalInput")
    cos_d = nc.dram_tensor("cosT", [128, S], BF16, kind="ExternalInput")
    sin_d = nc.dram_tensor("sinT", [128, S], BF16, kind="ExternalInput")
    mask_d = nc.dram_tensor("mask", [128, 128], BF16, kind="ExternalInput")
    sel_d = nc.dram_tensor("sel", [4, 256], BF16, kind="ExternalInput")
    out_d = nc.dram_tensor("out", [D, S], F32, kind="ExternalOutput")

    with tile.TileContext(nc) as tc:
        with (
            tc.tile_pool(name="const", bufs=1) as constp,
            tc.tile_pool(name="wpool", bufs=1) as wpool,
            tc.tile_pool(name="qkpool", bufs=1) as qkpool,
            tc.tile_pool(name="vpool", bufs=1) as vpool,
            tc.tile_pool(name="xpool", bufs=1) as xpool,
            tc.tile_pool(name="rpool", bufs=1) as rpool,
            tc.tile_pool(name="ptpool", bufs=1) as ptpool,
            tc.tile_pool(name="nrmpool", bufs=1) as nrmpool,
            tc.tile_pool(name="otpool", bufs=1) as otpool,
            tc.tile_pool(name="stp", bufs=1, space="PSUM") as stp,
            tc.tile_pool(name="cxp", bufs=1, space="PSUM") as cxp,
            tc.tile_pool(name="genp", bufs=1, space="PSUM") as genp,
        ):
            # ---- persistent weights / constants (parallel DMA queues) ----
            we = [wpool.tile([128, 1024], BF16, name=f"we{e}") for e in range(8)]
            wv = [wpool.tile([128, 512], BF16, name=f"wv{d}") for d in range(ND)]
            wo = [wpool.tile([128, D], BF16, name=f"wo{p}") for p in range(NPAIR)]
            for e in range(8):
                nc.gpsimd.dma_start(we[e][:], weT_d[e])
            for d in range(ND):
                nc.gpsimd.dma_start(wv[d][:], wvT_d[d])
            mask_sb = constp.tile([128, 128], BF16)
            sel_sb = constp.tile([4, 256], BF16)
            nc.gpsimd.dma_start(mask_sb[:], mask_d[:])
            nc.gpsimd.dma_start(sel_sb[:], sel_d[:])
            for p in range(NPAIR):
                nc.gpsimd.dma_start(wo[p][:], woT_d[p])

            qt = [qkpool.tile([128, S], BF16, name=f"qt{p}") for p in range(NPAIR)]
            kt = [qkpool.tile([128, S], BF16, name=f"kt{p}") for p in range(NPAIR)]
            vt = [vpool.tile([128, HLOC, DH + 1], BF16, name=f"v{t}")
                  for t in range(4 * NT)]
            for t in range(4 * NT):
                nc.gpsimd.memset(vt[t][:, :, DH:DH + 1], 1.0)

            def emit_loads(ts):
                tsl = slice(ts * 512, (ts + 1) * 512)
                xa = xpool.tile([128, ND, 512], BF16, tag="x", bufs=2,
                                name=f"xa{ts}")
                for d in range(ND):
                    # first tile: split across sync + scalar queues (no exp
                    # traffic exists yet) so the first chain starts ~1us in
                    eng = nc.scalar if (ts == 0 and d % 2 == 1) else nc.sync
                    eng.dma_start(xa[:, d, :], xT_d[d * 128:(d + 1) * 128, tsl])
                cos_sb = rpool.tile([128, 512], BF16, tag="cos", bufs=2,
                                    name=f"cos{ts}")
                sin_sb = rpool.tile([128, 512], BF16, tag="sin", bufs=2,
                                    name=f"sin{ts}")
                nc.sync.dma_start(cos_sb[:], cos_d[:, tsl])
                nc.sync.dma_start(sin_sb[:], sin_d[:, tsl])
                return xa, cos_sb, sin_sb

            deng = None
            loads = emit_loads(0)
            for ts in range(NT):
                tsl = slice(ts * 512, (ts + 1) * 512)
                i = ts
                isl = tsl
                xa, cos_sb, sin_sb = loads

                # Q^T / K^T e-chunks with RoPE
                for e in range(8):
                    ps = genp.tile([128, 512], F32, tag="gen", bufs=2)
                    for d in range(ND):
                        nc.tensor.matmul(
                            ps[:], we[e][:, d * 128:(d + 1) * 128], xa[:, d, :],
                            start=(d == 0), stop=(d == ND - 1),
                        )
                    dst = qt[e] if e < 4 else kt[e - 4]
                    nc.vector.tensor_copy(dst[:, tsl], ps[:])
                    # RoPE: quadrant swap via SBUF->SBUF DMA, then 2 muls + add
                    sw = rpool.tile([128, 512], BF16, tag="sw", bufs=3)
                    for qd in range(4):
                        sq = qd ^ 1
                        nc.gpsimd.dma_start(
                            sw[qd * 32:(qd + 1) * 32, :],
                            dst[sq * 32:(sq + 1) * 32, tsl],
                        )
                    t1 = rpool.tile([128, 512], BF16, tag="t1", bufs=3)
                    nc.vector.tensor_mul(t1[:], dst[:, tsl], cos_sb[:])
                    nc.vector.tensor_mul(sw[:], sw[:], sin_sb[:])
                    nc.vector.tensor_add(dst[:, tsl], t1[:], sw[:])

                # V chunks (natural [t, h, dv] layout, ones column appended)
                for tq0 in range(4):
                    tq = ts * 4 + tq0
                    psv = genp.tile([128, 512], F32, tag="gen", bufs=2)
                    for d in range(ND):
                        nc.tensor.matmul(
                            psv[:], xa[:, d, tq0 * 128:(tq0 + 1) * 128], wv[d][:],
                            start=(d == 0), stop=(d == ND - 1),
                        )
                    nc.vector.tensor_copy(
                        vt[tq][:, :, 0:DH],
                        psv.rearrange("p (h d) -> p h d", h=HLOC),
                    )

                if ts + 1 < NT:
                    loads = emit_loads(ts + 1)

                # ---------------- attention row i=ts ----------------
                nj = 4 * i + 4
                for p in range(NPAIR):
                    cxa = cxp.tile([65, 512], F32, tag="cxa", bufs=1,
                                   name=f"cxa{i}_{p}")
                    cxb = cxp.tile([65, 512], F32, tag="cxb", bufs=1,
                                   name=f"cxb{i}_{p}")
                    for j in range(nj):
                        lo = max(0, 128 * j - 512 * i)
                        qsl = slice(512 * i + lo, 512 * (i + 1))
                        ksl = slice(j * 128, (j + 1) * 128)
                        st = stp.tile([128, 2, 512], F32, tag="st", bufs=2)
                        nc.tensor.matmul(
                            st[:, 0, lo:512], kt[p][0:64, ksl],
                            qt[p][0:64, qsl], tile_position=(0, 0),
                        )
                        nc.tensor.matmul(
                            st[:, 1, lo:512], kt[p][64:128, ksl],
                            qt[p][64:128, qsl], tile_position=(64, 0),
                        )
                        pt = ptpool.tile([128, 2, 512], BF16, tag="pt", bufs=6)
                        nc.scalar.activation(
                            pt[:, :, lo:512], st[:, :, lo:512], EXP, scale=SCALE)
                        if lo == 128 * j - 512 * i:  # block straddles diagonal
                            nc.vector.tensor_mul(
                                pt[:, :, lo:lo + 128],
                                pt[:, :, lo:lo + 128],
                                mask_sb[:, None, :].to_broadcast([128, 2, 128]),
                            )
                        nc.tensor.matmul(
                            cxa[:, lo:512], vt[j][:, 2 * p, :],
                            pt[:, 0, lo:512],
                            start=(j == 0), stop=(j == nj - 1),
                        )
                        nc.tensor.matmul(
                            cxb[:, lo:512], vt[j][:, 2 * p + 1, :],
                            pt[:, 1, lo:512],
                            start=(j == 0), stop=(j == nj - 1),
                        )

                    # stash (row 64 = softmax denominator), gather, repack
                    g, q = p // 2, p % 2
                    if q == 0:
                        deng = nrmpool.tile([4, 512], BF16, tag="deng", bufs=2,
                                            name=f"deng{i}_{g}")
                    ctxb_sb = nrmpool.tile([65, 512], BF16, tag="ctxb", bufs=2)
                    nc.vector.tensor_copy(qt[p][0:65, isl], cxa[:])
                    if p == NPAIR - 1:
                        nc.scalar.copy(ctxb_sb[:], cxb[:])
                    else:
                        nc.vector.tensor_copy(ctxb_sb[:], cxb[:])
                    nc.gpsimd.dma_start(deng[2 * q:2 * q + 1, :],
                                        qt[p][64:65, isl])
                    nc.gpsimd.dma_start(
                        deng[2 * q + 1:2 * q + 2, :], ctxb_sb[64:65, :])
                    nc.gpsimd.dma_start(qt[p][64:128, isl], ctxb_sb[0:64, :])

                    if q == 1:  # pairs 2g, 2g+1 stashed -> normalize them
                        denf = nrmpool.tile([4, 512], F32, tag="denf", bufs=2)
                        nc.vector.tensor_copy(denf[:], deng[:])
                        rec = nrmpool.tile([4, 512], F32, tag="rec", bufs=2)
                        nc.vector.reciprocal_approx_fast(rec[:], denf[:])
                        recr = nrmpool.tile([4, 512], BF16, tag="recr", bufs=2)
                        nc.vector.tensor_copy(recr[:], rec[:])
                        for pp in (2 * g, 2 * g + 1):
                            qq = pp % 2
                            bc = cxp.tile([128, 512], F32,
                                          tag=("cxa" if qq == 0 else "cxb"),
                                          bufs=1, name=f"bc{i}_{pp}")
                            nc.tensor.matmul(
                                bc[:], sel_sb[:, qq * 128:(qq + 1) * 128],
                                recr[:])
                            bcs = nrmpool.tile([128, 512], BF16, tag="bcs",
                                               bufs=2)
                            nc.vector.tensor_copy(bcs[:], bc[:])
                            nc.vector.tensor_mul(
                                qt[pp][:, isl], qt[pp][:, isl], bcs[:])

                # ---------------- out-projection row i=ts ----------------
                for ec in range(D // 128):
                    ecs = slice(ec * 128, (ec + 1) * 128)
                    pso = cxp.tile([128, 512], F32,
                                   tag=("cxa" if ec % 2 == 0 else "cxb"),
                                   bufs=1, name=f"pso{i}_{ec}")
                    for p in range(NPAIR):
                        nc.tensor.matmul(
                            pso[:], wo[p][:, ecs], qt[p][:, isl],
                            start=(p == 0), stop=(p == NPAIR - 1),
                        )
                    ot = otpool.tile([128, 512], F32, tag="ot", bufs=3)
                    if i < 3:
                        nc.vector.tensor_copy(ot[:], pso[:])
                    else:
                        nc.scalar.copy(ot[:], pso[:])
                    nc.gpsimd.dma_start(out_d[ecs, isl], ot[:])

    nc.compile()
    return nc


def _get_program():
    global _PROGRAM
    if _PROGRAM is None:
        _PROGRAM = _build_program()
    return _PROGRAM


def _prep_in_maps(in_features, token_positions, W_qkv, W_out):
    in_features = np.asarray(in_features, dtype=np.float32)
    token_positions = np.asarray(token_positions)
    W_qkv = np.asarray(W_qkv, dtype=np.float32)
    W_out = np.asarray(W_out, dtype=np.float32)

    # RoPE pair permutation: [x0 of freq 0..31 | x1 of freq 0..31]
    perm = np.concatenate([np.arange(0, DH, 2), np.arange(1, DH, 2)])

    weT, wvT, woT = [], [], []
    for tp in range(TP):
        rows = []
        for sect in range(2):  # Q, K (permuted)
            for h in range(HLOC):
                g = tp * HLOC + h
                rows.append(W_qkv[sect * D + g * DH + perm])
        for h in range(HLOC):  # V natural
            g = tp * HLOC + h
            rows.append(W_qkv[2 * D + g * DH:2 * D + (g + 1) * DH])
        Wl = np.concatenate(rows, axis=0)          # [1536, 1024]
        wqkvT = np.ascontiguousarray(Wl.T)         # [1024, 1536]
        tmp = wqkvT[:, 0:1024].reshape(8, 128, 8, 128)        # [d, p, e, c]
        weT.append(np.ascontiguousarray(
            tmp.transpose(2, 1, 0, 3).reshape(8, 128, 1024).astype(BF)))
        wvT.append(np.ascontiguousarray(
            wqkvT[:, 1024:1536].reshape(8, 128, 512).astype(BF)))
        woT.append(np.ascontiguousarray(np.stack(
            [np.concatenate([
                W_out[:, (tp * HLOC + 2 * p) * DH:(tp * HLOC + 2 * p + 1) * DH].T,
                W_out[:, (tp * HLOC + 2 * p + 1) * DH:(tp * HLOC + 2 * p + 2) * DH].T,
            ], axis=0) for p in range(NPAIR)])).astype(BF))

    half = DH // 2
    inv_freq = (THETA ** (-2.0 * np.arange(half, dtype=np.float32) / DH)
                ).astype(np.float32)
    ang = token_positions.astype(np.float32)[:, None] * inv_freq[None, :]
    cos_t = np.cos(ang).T.astype(np.float32)  # [32, S]
    sin_t = np.sin(ang).T.astype(np.float32)
    cos128 = np.ascontiguousarray(np.tile(cos_t, (4, 1))).astype(BF)
    sin128 = np.ascontiguousarray(
        np.tile(np.concatenate([-sin_t, sin_t], axis=0), (2, 1))).astype(BF)
    # mask[kv, q] = 1 iff kv <= q (scores stored transposed: [kv, q])
    mask128 = np.triu(np.ones((128, 128), dtype=np.float32)).astype(BF)
    # sel[r, q*128 + c] = 1 iff r == 2q + (c >= 64): pair-packed recip bcast
    sel = np.zeros((4, 256), dtype=np.float32)
    for q in range(2):
        sel[2 * q, q * 128:q * 128 + 64] = 1.0
        sel[2 * q + 1, q * 128 + 64:(q + 1) * 128] = 1.0
    sel = sel.astype(BF)

    in_maps = []
    for c in range(NCORES):
        b, tp = c // 2, c % 2
        in_maps.append({
            "xT": np.ascontiguousarray(in_features[b].T).astype(BF),
            "weT": weT[tp],
            "wvT": wvT[tp],
            "woT": woT[tp],
            "cosT": cos128,
            "sinT": sin128,
            "mask": mask128,
            "sel": sel,
        })
    return in_maps


def run(in_features, token_positions, W_qkv, W_out, **spmd_kwargs):
    """Run the kernel; returns (output [B,S,D] f32, BassKernelResults)."""
    in_maps = _prep_in_maps(in_features, token_positions, W_qkv, W_out)
    nc = _get_program()
    res = run_bass_kernel_spmd(nc, in_maps, core_ids=list(range(NCORES)),
                               **spmd_kwargs)
    outs = [res.results[c]["out"] for c in range(NCORES)]
    full = np.stack([(outs[2 * b] + outs[2 * b + 1]).T for b in range(B)])
    return full.astype(np.float32), res


def kernel(in_features, token_positions, W_qkv, W_out):
    out, _ = run(in_features, token_positions, W_qkv, W_out)
    return out


# revision 8
# speedup vs baseline: 1.1317x; 1.1113x over previous
"""Multi-head self-attention (RoPE, causal) TRN2 Bass kernel — fused v2.

Problem: B=4, S=2048, D=1024, H=16, Dh=64, fp32 in/out.

Sharding (8 cores): DP=4 over batch x TP=2 over heads (Megatron-style).
Core c handles batch c//2 with heads (c%2)*8 .. (c%2)*8+7 and produces a
partial output [D, S]; the host sums the two TP partials per batch (the
all-reduce after out_projection) and transposes back.

v2 changes vs the 3-phase baseline (389us):
  - Everything bf16 on SBUF (same 1 cyc/row PE speed as f32r, 2x DVE modes,
    half the SBUF/DMA footprint, FWL on LDWEIGHTS). PSUM stays f32.
  - Single fused pipeline: per 512-token tile ts, emit QKV projection(ts),
    then causal attention row i=ts (kv tiles 0..ts), then out-projection of
    row ts. Tile's cost-model scheduler interleaves across sections, so the
    PE never idles long enough to drop the HAM clock gate, and the
    ACT-bound attention inner loop is overlapped with QKV/out-proj matmuls.
  - ACT (scalar engine) runs ONLY the softmax exps (the true ACT floor);
    all copies are pinned to DVE explicitly.
  - e-major W layout + per-d X chunks so the first matmul can start ~1us in.
  - Normalization: per-2-pair-group reciprocal, pair-packed one-hot
    broadcast matmuls, dedicated PSUM rotation, den gathers issued per-pair.
"""

import sys

for _p in ("/opt/trn_rl_repo", "/root/.axon_site/_ro/trn_rl_repo"):
    if _p not in sys.path:
        sys.path.insert(0, _p)

import ml_dtypes
import numpy as np

import concourse.bacc as bacc
import concourse.bass_utils as bass_utils
import concourse.mybir as mybir
import concourse.tile as tile
from concourse.bass_utils import run_bass_kernel_spmd

F32 = mybir.dt.float32
BF16 = mybir.dt.bfloat16
EXP = mybir.ActivationFunctionType.Exp
BF = ml_dtypes.bfloat16

B, S, D = 4, 2048, 1024
H, DH = 16, 64
THETA = 10000.0
NCORES, TP, HLOC = 8, 2, 8
NPAIR = HLOC // 2
NT = S // 512                        # 4 tiles of 512 tokens
ND = D // 128                        # 8 contraction chunks
SCALE = 1.0 / 8.0                    # 1/sqrt(DH)

_PROGRAM = None


def _build_program():
    nc = bacc.Bacc(None)

    xT_d = nc.dram_tensor("xT", [D, S], BF16, kind="ExternalInput")
    weT_d = nc.dram_tensor("weT", [8, 128, 1024], BF16, kind="ExternalInput")
    wvT_d = nc.dram_tensor("wvT", [8, 128, 512], BF16, kind="ExternalInput")
    woT_d = nc.dram_tensor("woT", [NPAIR, 128, D], BF16, kind="ExternalInput")
    cos_d = nc.dram_tensor("cosT", [128, S], BF16, kind="ExternalInput")
    sin_d = nc.dram_tensor("sinT", [128, S], BF16, kind="ExternalInput")
    mask_d = nc.dram_tensor("mask", [128, 128], BF16, kind="ExternalInput")
    sel_d = nc.dram_tensor("sel", [4, 256], BF16, kind="ExternalInput")
    perm_d = nc.dram_tensor("permM", [128, 128], BF16, kind="ExternalInput")
    out_d = nc.dram_tensor("out", [D, S], F32, kind="ExternalOutput")

    with tile.TileContext(nc) as tc:
        with (
            tc.tile_pool(name="const", bufs=1) as constp,
            tc.tile_pool(name="wpool", bufs=1) as wpool,
            tc.tile_pool(name="qkpool", bufs=1) as qkpool,
            tc.tile_pool(name="vpool", bufs=1) as vpool,
            tc.tile_pool(name="xpool", bufs=1) as xpool,
            tc.tile_pool(name="rpool", bufs=1) as rpool,
            tc.tile_pool(name="ptpool", bufs=1) as ptpool,
            tc.tile_pool(name="nrmpool", bufs=1) as nrmpool,
            tc.tile_pool(name="otpool", bufs=1) as otpool,
            tc.tile_pool(name="stp", bufs=1, space="PSUM") as stp,
            tc.tile_pool(name="cxp", bufs=1, space="PSUM") as cxp,
            tc.tile_pool(name="genp", bufs=1, space="PSUM") as genp,
        ):
            # ---- persistent weights / constants (parallel DMA queues) ----
            we = [wpool.tile([128, 1024], BF16, name=f"we{e}") for e in range(8)]
            wv = [wpool.tile([128, 512], BF16, name=f"wv{d}") for d in range(ND)]
            wo = [wpool.tile([128, D], BF16, name=f"wo{p}") for p in range(NPAIR)]
            for e in range(8):
                nc.gpsimd.dma_start(we[e][:], weT_d[e])
            for d in range(ND):
                nc.gpsimd.dma_start(wv[d][:], wvT_d[d])
            mask_sb = constp.tile([128, 128], BF16)
            sel_sb = constp.tile([4, 256], BF16)
            perm_sb = constp.tile([128, 128], BF16)
            nc.gpsimd.dma_start(perm_sb[:], perm_d[:])
            nc.gpsimd.dma_start(mask_sb[:], mask_d[:])
            nc.gpsimd.dma_start(sel_sb[:], sel_d[:])
            for p in range(NPAIR):
                nc.gpsimd.dma_start(wo[p][:], woT_d[p])

            qt = [qkpool.tile([128, S], BF16, name=f"qt{p}") for p in range(NPAIR)]
            kt = [qkpool.tile([128, S], BF16, name=f"kt{p}") for p in range(NPAIR)]
            vt = [vpool.tile([128, HLOC, DH + 1], BF16, name=f"v{t}")
                  for t in range(4 * NT)]
            for t in range(4 * NT):
                nc.gpsimd.memset(vt[t][:, :, DH:DH + 1], 1.0)

            def emit_loads(ts):
                tsl = slice(ts * 512, (ts + 1) * 512)
                xa = xpool.tile([128, ND, 512], BF16, tag="x", bufs=2,
                                name=f"xa{ts}")
                for d in range(ND):
                    # first tile: split across sync + scalar queues (no exp
                    # traffic exists yet) so the first chain starts ~1us in
                    eng = nc.scalar if (ts == 0 and d % 2 == 1) else nc.sync
                    eng.dma_start(xa[:, d, :], xT_d[d * 128:(d + 1) * 128, tsl])
                cos_sb = rpool.tile([128, 512], BF16, tag="cos", bufs=2,
                                    name=f"cos{ts}")
                sin_sb = rpool.tile([128, 512], BF16, tag="sin", bufs=2,
                                    name=f"sin{ts}")
                nc.sync.dma_start(cos_sb[:], cos_d[:, tsl])
                nc.sync.dma_start(sin_sb[:], sin_d[:, tsl])
                return xa, cos_sb, sin_sb

            deng = None
            loads = emit_loads(0)
            for ts in range(NT):
                tsl = slice(ts * 512, (ts + 1) * 512)
                i = ts
                isl = tsl
                xa, cos_sb, sin_sb = loads

                # Q^T / K^T e-chunks with RoPE
                for e in range(8):
                    ps = genp.tile([128, 512], F32, tag="gen", bufs=2)
                    for d in range(ND):
                        nc.tensor.matmul(
                            ps[:], we[e][:, d * 128:(d + 1) * 128], xa[:, d, :],
                            start=(d == 0), stop=(d == ND - 1),
                        )
                    dst = qt[e] if e < 4 else kt[e - 4]
                    dtmp = rpool.tile([128, 512], BF16, tag="dtmp", bufs=3)
                    nc.vector.tensor_copy(dtmp[:], ps[:])
                    # RoPE: pair-swap (sign folded into permM) via PE matmul.
                    # dst is write-only here (no in-place WAR with the PE
                    # streaming read).
                    swp = genp.tile([128, 512], F32, tag="gen", bufs=2,
                                    name=f"swp{ts}_{e}")
                    nc.tensor.matmul(swp[:], perm_sb[:], dtmp[:])
                    sw = rpool.tile([128, 512], BF16, tag="sw", bufs=3)
                    nc.vector.tensor_mul(sw[:], swp[:], sin_sb[:])
                    t1 = rpool.tile([128, 512], BF16, tag="t1", bufs=3)
                    nc.vector.tensor_mul(t1[:], dtmp[:], cos_sb[:])
                    nc.gpsimd.tensor_add(dst[:, tsl], t1[:], sw[:])

                # V chunks (natural [t, h, dv] layout, ones column appended)
                for tq0 in range(4):
                    tq = ts * 4 + tq0
                    psv = genp.tile([128, 512], F32, tag="gen", bufs=2)
                    for d in range(ND):
                        nc.tensor.matmul(
                            psv[:], xa[:, d, tq0 * 128:(tq0 + 1) * 128], wv[d][:],
                            start=(d == 0), stop=(d == ND - 1),
                        )
                    nc.vector.tensor_copy(
                        vt[tq][:, :, 0:DH],
                        psv.rearrange("p (h d) -> p h d", h=HLOC),
                    )

                if ts + 1 < NT:
                    loads = emit_loads(ts + 1)

                # ---------------- attention row i=ts ----------------
                nj = 4 * i + 4
                for p in range(NPAIR):
                    cxa = cxp.tile([65, 512], F32, tag="cxa", bufs=1,
                                   name=f"cxa{i}_{p}")
                    cxb = cxp.tile([65, 512], F32, tag="cxb", bufs=1,
                                   name=f"cxb{i}_{p}")
                    for j in range(nj):
                        lo = max(0, 128 * j - 512 * i)
                        qsl = slice(512 * i + lo, 512 * (i + 1))
                        ksl = slice(j * 128, (j + 1) * 128)
                        st = stp.tile([128, 2, 512], F32, tag="st", bufs=2)
                        nc.tensor.matmul(
                            st[:, 0, lo:512], kt[p][0:64, ksl],
                            qt[p][0:64, qsl], tile_position=(0, 0),
                        )
                        nc.tensor.matmul(
                            st[:, 1, lo:512], kt[p][64:128, ksl],
                            qt[p][64:128, qsl], tile_position=(64, 0),
                        )
                        pt = ptpool.tile([128, 2, 512], BF16, tag="pt", bufs=6)
                        nc.scalar.activation(
                            pt[:, :, lo:512], st[:, :, lo:512], EXP, scale=SCALE)
                        if lo == 128 * j - 512 * i:  # block straddles diagonal
                            nc.vector.tensor_mul(
                                pt[:, :, lo:lo + 128],
                                pt[:, :, lo:lo + 128],
                                mask_sb[:, None, :].to_broadcast([128, 2, 128]),
                            )
                        nc.tensor.matmul(
                            cxa[:, lo:512], vt[j][:, 2 * p, :],
                            pt[:, 0, lo:512],
                            start=(j == 0), stop=(j == nj - 1),
                        )
                        nc.tensor.matmul(
                            cxb[:, lo:512], vt[j][:, 2 * p + 1, :],
                            pt[:, 1, lo:512],
                            start=(j == 0), stop=(j == nj - 1),
                        )

                    # stash (row 64 = softmax denominator), gather, repack
                    g, q = p // 2, p % 2
                    if q == 0:
                        deng = nrmpool.tile([4, 512], BF16, tag="deng", bufs=2,
                                            name=f"deng{i}_{g}")
                    ctxb_sb = nrmpool.tile([65, 512], BF16, tag="ctxb", bufs=2)
                    nc.vector.tensor_copy(qt[p][0:65, isl], cxa[:])
                    if p == NPAIR - 1:
                        nc.scalar.copy(ctxb_sb[:], cxb[:])
                    else:
                        nc.vector.tensor_copy(ctxb_sb[:], cxb[:])
                    nc.gpsimd.dma_start(deng[2 * q:2 * q + 1, :],
                                        qt[p][64:65, isl])
                    nc.gpsimd.dma_start(
                        deng[2 * q + 1:2 * q + 2, :], ctxb_sb[64:65, :])
                    nc.gpsimd.dma_start(qt[p][64:128, isl], ctxb_sb[0:64, :])

                    if q == 1:  # pairs 2g, 2g+1 stashed -> normalize them
                        denf = nrmpool.tile([4, 512], F32, tag="denf", bufs=2)
                        nc.vector.tensor_copy(denf[:], deng[:])
                        rec = nrmpool.tile([4, 512], F32, tag="rec", bufs=2)
                        nc.vector.reciprocal_approx_fast(rec[:], denf[:])
                        recr = nrmpool.tile([4, 512], BF16, tag="recr", bufs=2)
                        nc.vector.tensor_copy(recr[:], rec[:])
                        for pp in (2 * g, 2 * g + 1):
                            qq = pp % 2
                            bc = cxp.tile([128, 512], F32,
                                          tag=("cxa" if qq == 0 else "cxb"),
                                          bufs=1, name=f"bc{i}_{pp}")
                            nc.tensor.matmul(
                                bc[:], sel_sb[:, qq * 128:(qq + 1) * 128],
                                recr[:])
                            bcs = nrmpool.tile([128, 512], BF16, tag="bcs",
                                               bufs=2)
                            nc.vector.tensor_copy(bcs[:], bc[:])
                            nc.vector.tensor_mul(
                                qt[pp][:, isl], qt[pp][:, isl], bcs[:])

                # ---------------- out-projection row i=ts ----------------
                for ec in range(D // 128):
                    ecs = slice(ec * 128, (ec + 1) * 128)
                    pso = cxp.tile([128, 512], F32,
                                   tag=("cxa" if ec % 2 == 0 else "cxb"),
                                   bufs=1, name=f"pso{i}_{ec}")
                    for p in range(NPAIR):
                        nc.tensor.matmul(
                            pso[:], wo[p][:, ecs], qt[p][:, isl],
                            start=(p == 0), stop=(p == NPAIR - 1),
                        )
                    ot = otpool.tile([128, 512], F32, tag="ot", bufs=3)
                    if i < 3:
                        nc.vector.tensor_copy(ot[:], pso[:])
                    else:
                        nc.scalar.copy(ot[:], pso[:])
                    nc.sync.dma_start(out_d[ecs, isl], ot[:])

    nc.compile()
    return nc


def _get_program():
    global _PROGRAM
    if _PROGRAM is None:
        _PROGRAM = _build_program()
    return _PROGRAM


def _prep_in_maps(in_features, token_positions, W_qkv, W_out):
    in_features = np.asarray(in_features, dtype=np.float32)
    token_positions = np.asarray(token_positions)
    W_qkv = np.asarray(W_qkv, dtype=np.float32)
    W_out = np.asarray(W_out, dtype=np.float32)

    # RoPE pair permutation: [x0 of freq 0..31 | x1 of freq 0..31]
    perm = np.concatenate([np.arange(0, DH, 2), np.arange(1, DH, 2)])

    weT, wvT, woT = [], [], []
    for tp in range(TP):
        rows = []
        for sect in range(2):  # Q, K (permuted)
            for h in range(HLOC):
                g = tp * HLOC + h
                rows.append(W_qkv[sect * D + g * DH + perm])
        for h in range(HLOC):  # V natural
            g = tp * HLOC + h
            rows.append(W_qkv[2 * D + g * DH:2 * D + (g + 1) * DH])
        Wl = np.concatenate(rows, axis=0)          # [1536, 1024]
        wqkvT = np.ascontiguousarray(Wl.T)         # [1024, 1536]
        tmp = wqkvT[:, 0:1024].reshape(8, 128, 8, 128)        # [d, p, e, c]
        weT.append(np.ascontiguousarray(
            tmp.transpose(2, 1, 0, 3).reshape(8, 128, 1024).astype(BF)))
        wvT.append(np.ascontiguousarray(
            wqkvT[:, 1024:1536].reshape(8, 128, 512).astype(BF)))
        woT.append(np.ascontiguousarray(np.stack(
            [np.concatenate([
                W_out[:, (tp * HLOC + 2 * p) * DH:(tp * HLOC + 2 * p + 1) * DH].T,
                W_out[:, (tp * HLOC + 2 * p + 1) * DH:(tp * HLOC + 2 * p + 2) * DH].T,
            ], axis=0) for p in range(NPAIR)])).astype(BF))

    half = DH // 2
    inv_freq = (THETA ** (-2.0 * np.arange(half, dtype=np.float32) / DH)
                ).astype(np.float32)
    ang = token_positions.astype(np.float32)[:, None] * inv_freq[None, :]
    cos_t = np.cos(ang).T.astype(np.float32)  # [32, S]
    sin_t = np.sin(ang).T.astype(np.float32)
    cos128 = np.ascontiguousarray(np.tile(cos_t, (4, 1))).astype(BF)
    sin128 = np.ascontiguousarray(np.tile(sin_t, (4, 1))).astype(BF)
    permM = np.zeros((128, 128), dtype=np.float32)
    for blk in (0, 64):
        for r in range(32):
            permM[blk + 32 + r, blk + r] = -1.0   # sw[blk+r]    = -x1
            permM[blk + r, blk + 32 + r] = 1.0    # sw[blk+32+r] = +x0
    permM = permM.astype(BF)
    # mask[kv, q] = 1 iff kv <= q (scores stored transposed: [kv, q])
    mask128 = np.triu(np.ones((128, 128), dtype=np.float32)).astype(BF)
    # sel[r, q*128 + c] = 1 iff r == 2q + (c >= 64): pair-packed recip bcast
    sel = np.zeros((4, 256), dtype=np.float32)
    for q in range(2):
        sel[2 * q, q * 128:q * 128 + 64] = 1.0
        sel[2 * q + 1, q * 128 + 64:(q + 1) * 128] = 1.0
    sel = sel.astype(BF)

    in_maps = []
    for c in range(NCORES):
        b, tp = c // 2, c % 2
        in_maps.append({
            "xT": np.ascontiguousarray(in_features[b].T).astype(BF),
            "weT": weT[tp],
            "wvT": wvT[tp],
            "woT": woT[tp],
            "cosT": cos128,
            "sinT": sin128,
            "mask": mask128,
            "sel": sel,
            "permM": permM,
        })
    return in_maps


def run(in_features, token_positions, W_qkv, W_out, **spmd_kwargs):
    """Run the kernel; returns (output [B,S,D] f32, BassKernelResults)."""
    in_maps = _prep_in_maps(in_features, token_positions, W_qkv, W_out)
    nc = _get_program()
    res = run_bass_kernel_spmd(nc, in_maps, core_ids=list(range(NCORES)),
                               **spmd_kwargs)
    outs = [res.results[c]["out"] for c in range(NCORES)]
    full = np.stack([(outs[2 * b] + outs[2 * b + 1]).T for b in range(B)])
    return full.astype(np.float32), res


def kernel(in_features, token_positions, W_qkv, W_out):
    out, _ = run(in_features, token_positions, W_qkv, W_out)
    return out
